# revision 56
# baseline (speedup 1.0000x reference)
"""Trainium2 Bass kernel for the EnergyCoulomb problem.

Reference computation (per molecule, B=32, N=512, D=1024, H=512):
  y  = sum_atoms(mask * (ssp(rep @ W1 + b1) @ W2 + b2))           atomwise MLP + pool
  q  = ssp(rep @ Wc1 + bc1) @ Wc2 + bc2                           charge net
  e  = sum_{i!=j} q_i q_j (1e-5 + |R_i - R_j|)^-2 * mask_i mask_j coulomb term
  out = y + e

Sharding: data-parallel over molecules, 4 molecules per core on 8 cores,
weights replicated.

Key design points (vs the 131.7us first-generation kernel):
  * rep is pre-transposed on the host into [128, KD*N] per molecule: the PE
    never transposes it and nothing copies transposes out of PSUM.
  * The cost model's DMA pool is one serial ~330GB/s resource, so transfers
    are emitted in consumption order in 2KB-per-partition k-chunks, and the
    z matmuls run K-MAJOR per molecule: every arriving chunk immediately
    enables 4 matmuls (one per h-chunk), so the PE streams at DMA rate
    through the startup phase instead of stalling per group.
  * (1e-5 + dist)^-2 is approximated by 1/d2 (max rel err ~8e-4 on the
    closest pairs, far under the 2e-2 gate): the entire sqrt chain
    (Ln, Exp, +1e-5, square) disappears. The diagonal d2 is exactly zero
    by construction; reciprocal gives inf there and affine_select
    replaces it with 0 before any consumer.
  * R rows and mask rows are built ON DEVICE (PE column->row transposes +
    Pool partition_broadcast) from column-spread inputs, keeping row
    broadcasts out of the serial DMA stream.
  * d2 squares: x,y coords on ACT (Square with bias=-coord), z coord on
    Pool(sub)+DVE(mul) to balance engines; the softplus Ln stage is fused
    to one [128, HC*N] instruction per (molecule, set).
  * All pairwise chains and row builds are emitted early so the per-
    molecule coulomb epilogues never wait on them.

ssp(x) = softplus(x) - ln2 is folded as softplus on device plus a host-side
constant shift c = b - ln2 * sum(W_layer2) applied at the pooled level.
"""

import numpy as np

import concourse.bass as bass
import concourse.bacc as bacc
import concourse.mybir as mybir
import concourse.tile as tile
from concourse import bass_utils
from concourse.masks import make_identity

LOG2 = float(np.log(2.0))

B, N, D, H = 32, 512, 1024, 512
NCORES = 8
BL = B // NCORES          # molecules per core
P = 128                   # partitions
KD = D // P               # 8 K-chunks over D
HC = H // P               # 4 h-chunks over H
IC = N // P               # 4 i-chunks over atoms

f32 = mybir.dt.float32
f32r = mybir.dt.float32r
AF = mybir.ActivationFunctionType
ALU = mybir.AluOpType
AX = mybir.AxisListType

_CACHE = {}

# Every ACT function this kernel uses (Exp, Ln, Square, Copy) lives in the
# "natural_log_exp_and_others" table set. Bacc's table chooser is
# greedy-first-match; emptying every other set (order preserved, so
# act_func_set_id indices stay valid) pins the chooser to the combined set:
# one table load for the whole kernel.
_ONE_TABLE = "natural_log_exp_and_others"


def _gat_one_table(arch):
    from concourse.hw_specs import get_activation_tables
    tabs = get_activation_tables(arch)
    assert _ONE_TABLE in tabs
    return {n: (fns if n == _ONE_TABLE else set()) for n, fns in tabs.items()}


def _build_program():
    bacc.get_activation_tables = _gat_one_table
    nc = bacc.Bacc("TRN2", target_bir_lowering=False, debug=False,
                   enable_asserts=False)

    # host-pretransposed rep: rt[b][p, k*N+n] = rep[b, n, k*128+p]
    rept_d = nc.dram_tensor("rept", [BL, P, KD * N], f32r, kind="ExternalInput").ap()
    # host-chunked weights: w1h[p, k*H+h] = W1[k*128+p, h]
    w1h_d = nc.dram_tensor("w1h", [P, KD * H], f32r, kind="ExternalInput").ap()
    wc1h_d = nc.dram_tensor("wc1h", [P, KD * H], f32r, kind="ExternalInput").ap()
    # all small inputs packed into two tensors (each DMA trigger costs
    # ~630ns of ring-sequencer time, so 8 separate smalls would delay the
    # weight stream by ~4us): packed f32 columns are
    #   rcoln[48] | rcolp[48] | maskc[16] | b1t[4] | bc1t[4] | cvec row[5]
    # where rcoln[p, (b*IC+ic)*3+c] = -R[b, ic*128+p, c] (bias/scalar
    # operands), rcolp the same un-negated (row-transpose source), and
    # maskc[p, b*IC+ic] = mask[b, ic*128+p].
    NPK = BL * IC * 3 * 2 + BL * IC + 2 * HC + (BL + 1)
    pack_d = nc.dram_tensor("pack", [P, NPK], f32, kind="ExternalInput").ap()
    packr_d = nc.dram_tensor("packr", [P, 2 * HC], f32r, kind="ExternalInput").ap()
    out_d = nc.dram_tensor("out", [1, BL], f32, kind="ExternalOutput").ap()

    with tile.TileContext(nc) as tc:
        with tc.tile_pool(name="singles", bufs=1) as singles, \
             tc.tile_pool(name="work", bufs=1) as work, \
             tc.tile_pool(name="ps", bufs=1, space="PSUM") as ps:

            ident32 = singles.tile([1, 1], f32, tag="ident32")
            nc.vector.memset(ident32, 1.0)
            ident = singles.tile([P, P], f32, tag="ident")
            make_identity(nc, ident)
            ones_col = singles.tile([P, 1], f32, tag="ones_col")
            nc.vector.memset(ones_col, 1.0)

            # ---- input streaming ----
            # Everything rides the SP-sequencer HWDGE ring in consumption
            # order: each trigger costs ~630ns of its host sequencer, so
            # putting any of these on the ACT ring would clog the ACT
            # engine's instruction queue. 4KB chunks keep the trigger rate
            # comfortably ahead of the ~330GB/s serial transfer rate while
            # still drip-feeding the k-major matmuls.
            def ring_dma(dst, src):
                nc.sync.dma_start(dst, src)

            rt_sb = []
            for b in range(BL):
                rt_sb.append(work.tile([P, KD * N], f32r, tag="rt", bufs=BL - 1,
                                       name=f"rt{b}"))
            wc1_sb = singles.tile([P, KD * H], f32r, tag="wc1h")
            w1_sb = singles.tile([P, KD * H], f32r, tag="w1h")

            pack = singles.tile([P, NPK], f32, tag="pack")
            nc.sync.dma_start(pack, pack_d)
            packr = singles.tile([P, 2 * HC], f32r, tag="packr")
            nc.sync.dma_start(packr, packr_d)
            NC3 = BL * IC * 3
            rcoln = pack[:, 0:NC3]
            rcolp = pack[:, NC3:2 * NC3]
            maskc = pack[:, 2 * NC3:2 * NC3 + BL * IC]
            _o = 2 * NC3 + BL * IC
            b1t = pack[:, _o:_o + HC]
            bc1t = pack[:, _o + HC:_o + 2 * HC]
            cvec = pack[0:1, _o + 2 * HC:_o + 2 * HC + BL + 1]
            w2t = packr[:, 0:HC]
            wc2t = packr[:, HC:2 * HC]

            # per-molecule consumption order: wc1 (mol0 charge), rt0, w1
            # (mol0 y), rt1, rt2, rt3 — in 2-k (4KB/partition) chunks
            for k in range(0, KD, 2):
                ring_dma(wc1_sb[:, k * H:(k + 2) * H], wc1h_d[:, k * H:(k + 2) * H])
            for k in range(0, KD, 2):
                ring_dma(rt_sb[0][:, k * N:(k + 2) * N], rept_d[0][:, k * N:(k + 2) * N])
            for k in range(0, KD, 2):
                ring_dma(w1_sb[:, k * H:(k + 2) * H], w1h_d[:, k * H:(k + 2) * H])
            for b in range(1, BL):
                for k in range(0, KD, 2):
                    ring_dma(rt_sb[b][:, k * N:(k + 2) * N],
                             rept_d[b][:, k * N:(k + 2) * N])

            res = singles.tile([1, BL], f32, tag="res")

            # ---- device-side row builds (replaces row-broadcast DMAs) ----
            xjb_tiles = {}
            mrows = {}

            def build_xjb(b):
                # GPSIMD cannot read PSUM: transpose -> DVE copy to an SBUF
                # row -> Pool broadcast
                xjb = work.tile([P, 3, N], f32, tag="xjb", bufs=2)
                xrow = work.tile([1, 3, N], f32, tag="xrow", bufs=1)
                for c in range(3):
                    xp = ps.tile([1, N], f32, tag="xrow_ps", bufs=1)
                    for ic in range(IC):
                        nc.tensor.transpose(
                            xp[0:1, ic * P:(ic + 1) * P],
                            rcolp[:, (b * IC + ic) * 3 + c:(b * IC + ic) * 3 + c + 1],
                            ident)
                    nc.vector.tensor_copy(xrow[0:1, c, :], xp)
                nc.gpsimd.partition_broadcast(xjb, xrow)
                xjb_tiles[b] = xjb

            def build_mrow(b):
                mp = ps.tile([1, N], f32, tag="xrow_ps", bufs=1)
                for ic in range(IC):
                    nc.tensor.transpose(
                        mp[0:1, ic * P:(ic + 1) * P],
                        maskc[:, b * IC + ic:b * IC + ic + 1],
                        ident)
                m = singles.tile([1, N], f32, tag=f"mrow_{b}")
                nc.vector.tensor_copy(m, mp)
                mrows[b] = m

            # ---- pairwise chain: rb[p, ic, j] = 1/d2_(128ic+p),j  (0 diag) --
            # Split into an ACT-square part and a DVE/Pool part so each can
            # be placed independently in its engine's in-order queue.
            def chain_sq(b):
                xjb = xjb_tiles[b]
                d2b = work.tile([P, IC, N], f32, tag="d2b", bufs=1)
                tmpb = work.tile([P, IC, N], f32, tag="tmpb", bufs=1)
                for ic in range(IC):
                    col = (b * IC + ic) * 3
                    nc.scalar.activation(d2b[:, ic, :], xjb[:, 0, :], AF.Square,
                                         bias=rcoln[:, col + 0:col + 1])
                    nc.scalar.activation(tmpb[:, ic, :], xjb[:, 1, :], AF.Square,
                                         bias=rcoln[:, col + 1:col + 2])
                return d2b, tmpb

            def chain_rest(b, d2b, tmpb, dve_subs=False):
                xjb = xjb_tiles[b]
                nc.vector.tensor_tensor(d2b, d2b, tmpb, op=ALU.add)
                # z coord: subtract on Pool (DVE for the last chain, whose
                # latency is on the final epilogue's critical path), square +
                # accumulate on DVE
                sub_eng = nc.vector if dve_subs else nc.gpsimd
                for ic in range(IC):
                    col = (b * IC + ic) * 3
                    sub_eng.tensor_scalar(tmpb[:, ic, :], xjb[:, 2, :],
                                          rcoln[:, col + 2:col + 3], None,
                                          op0=ALU.add)
                nc.vector.tensor_mul(tmpb, tmpb, tmpb)
                nc.vector.tensor_tensor(d2b, d2b, tmpb, op=ALU.add)
                nc.vector.reciprocal(d2b, d2b)
                rb = work.tile([P, IC, N], f32r, tag="rb", bufs=2)
                nc.gpsimd.affine_select(
                    out=rb, in_=d2b, compare_op=ALU.not_equal, fill=0.0,
                    base=0, pattern=[[P, IC], [-1, N]], channel_multiplier=1)
                return rb

            rb_tiles = {}
            h_tiles = {}
            chain_mid = {}

            # ---- per-molecule MLP set, two phases ----
            # z-phase: k-major z matmuls, then the Exps IMMEDIATELY (they
            # free the PSUM slots the next set's matmuls are waiting on).
            # ln-phase: the (fused, bias-free) Ln runs later, in the next
            # set's z-window, so it never blocks exps in the in-order ACT
            # queue.
            ez_tiles = {}

            def emit_mlp_z(b, wset):
                # half-sets (hc 0,1 then hc 2,3): the first half's exps fire
                # ~3.4us before the set's end, so the next set's matmuls wait
                # only on the second half's exps (halved boundary stall).
                w_sb, bias = (wc1_sb, bc1t) if wset == "q" else (w1_sb, b1t)
                zts = [ps.tile([P, N], f32, tag="z", bufs=5,
                               name=f"z_{b}_{wset}_{hc}") for hc in range(HC)]
                ez = work.tile([P, HC, N], f32, tag="ez", bufs=2)
                for hc in range(HC):
                    for k in range(KD):
                        nc.tensor.matmul(
                            zts[hc],
                            lhsT=w_sb[:, k * H + hc * P:k * H + (hc + 1) * P],
                            rhs=rt_sb[b][:, k * N:(k + 1) * N],
                            start=(k == 0), stop=(k == KD - 1))
                    nc.scalar.activation(ez[:, hc, :], zts[hc], AF.Exp,
                                         bias=bias[:, hc:hc + 1])
                ez_tiles[(b, wset)] = ez

            def emit_mlp_ln(b, wset):
                ez = ez_tiles.pop((b, wset))
                h = work.tile([P, HC, N], f32r, tag=f"h_{wset}", bufs=2)
                nc.scalar.activation(h, ez, AF.Ln, bias=ones_col[:, 0:1])
                h_tiles[(b, wset)] = h

            # ---- per-molecule epilogue (charge front / finish split) ----
            epi_front = {}

            def emit_epi_front(b):
                """q row + q columns — needs only ln(b, q) and the mask."""
                hq = h_tiles.pop((b, "q"))
                q_ps = ps.tile([1, N], f32, tag="row_ps", bufs=2)
                for hc in range(HC):
                    nc.tensor.matmul(q_ps,
                                     lhsT=wc2t[:, hc:hc + 1],
                                     rhs=hq[:, hc, :],
                                     start=(hc == 0), stop=(hc == HC - 1))
                qrow = work.tile([1, N], f32, tag="qrow", bufs=1)
                nc.vector.tensor_scalar(qrow, q_ps, cvec[0:1, BL:BL + 1], None,
                                        op0=ALU.add)
                nc.vector.tensor_mul(qrow, qrow, mrows[b])

                qc_ps = ps.tile([P, IC], f32, tag="row_ps", bufs=2)
                for ic in range(IC):
                    nc.tensor.transpose(qc_ps[:, ic:ic + 1],
                                        qrow[:, ic * P:(ic + 1) * P],
                                        ident32[0:1, 0:1])
                qc = work.tile([P, IC], f32r, tag="qc", bufs=2)
                nc.vector.tensor_copy(qc, qc_ps)
                epi_front[b] = (qrow, qc)

            def emit_epilogue(b):
                if b not in epi_front:
                    emit_epi_front(b)
                qrow, qc = epi_front.pop(b)
                mrow = mrows[b]
                rb = rb_tiles.pop(b)
                t_ps = ps.tile([1, N], f32, tag="row_ps", bufs=2)
                for ic in range(IC):
                    nc.tensor.matmul(t_ps,
                                     lhsT=qc[:, ic:ic + 1],
                                     rhs=rb[:, ic, :],
                                     start=(ic == 0), stop=(ic == IC - 1))
                scr_e = work.tile([1, N], f32, tag="scr", bufs=1)
                nc.vector.tensor_mul(scr_e, t_ps, qrow)
                e_sb = work.tile([1, 1], f32, tag="e_sb", bufs=2)
                nc.vector.reduce_sum(e_sb, scr_e, axis=AX.X)

                h1 = h_tiles.pop((b, "y"))
                yi_ps = ps.tile([1, N], f32, tag="row_ps", bufs=2)
                for hc in range(HC):
                    nc.tensor.matmul(yi_ps,
                                     lhsT=w2t[:, hc:hc + 1],
                                     rhs=h1[:, hc, :],
                                     start=(hc == 0), stop=(hc == HC - 1))
                scr_y = work.tile([1, N], f32, tag="scr", bufs=1)
                nc.vector.tensor_mul(scr_y, yi_ps, mrow)
                ysum = work.tile([1, 1], f32, tag="ysum", bufs=2)
                nc.vector.reduce_sum(ysum, scr_y, axis=AX.X)
                y_sb = work.tile([1, 1], f32, tag="y_sb", bufs=2)
                nc.vector.tensor_add(y_sb, ysum, cvec[0:1, b:b + 1])
                nc.vector.tensor_add(res[:, b:b + 1], y_sb, e_sb)

            # ---- schedule ----
            # Row builds + all four pairwise chains go first: their ACT
            # squares fill the DMA-drip phase where ACT is otherwise idle,
            # and every rb is ready long before its epilogue.
            build_xjb(0)
            chain_mid[0] = chain_sq(0)
            emit_mlp_z(0, "q")
            build_xjb(1)                      # PE transposes fill the boundary
            rb_tiles[0] = chain_rest(0, *chain_mid.pop(0))
            emit_mlp_z(0, "y")
            chain_mid[1] = chain_sq(1)
            emit_mlp_ln(0, "q")
            emit_mlp_z(1, "q")
            build_xjb(2)
            rb_tiles[1] = chain_rest(1, *chain_mid.pop(1))
            build_mrow(0)
            emit_epi_front(0)
            emit_mlp_ln(0, "y")
            emit_epilogue(0)
            emit_mlp_z(1, "y")
            chain_mid[2] = chain_sq(2)
            emit_mlp_ln(1, "q")
            emit_mlp_z(2, "q")
            build_xjb(3)
            rb_tiles[2] = chain_rest(2, *chain_mid.pop(2))
            build_mrow(1)
            emit_epi_front(1)
            emit_mlp_ln(1, "y")
            emit_epilogue(1)
            emit_mlp_z(2, "y")
            chain_mid[3] = chain_sq(3)
            emit_mlp_ln(2, "q")
            build_mrow(2)
            emit_epi_front(2)
            emit_mlp_z(3, "q")
            rb_tiles[3] = chain_rest(3, *chain_mid.pop(3))
            emit_mlp_ln(2, "y")
            emit_epilogue(2)
            emit_mlp_ln(3, "q")
            build_mrow(3)
            emit_epi_front(3)
            emit_mlp_z(3, "y")
            emit_mlp_ln(3, "y")
            emit_epilogue(3)

            nc.sync.dma_start(out_d, res)

    nc.compile()
    return nc


def _get_program():
    if "nc" not in _CACHE:
        _CACHE["nc"] = _build_program()
    return _CACHE["nc"]


def _host_prep(inputs):
    """Build per-core in_maps from full inputs."""
    rep = np.asarray(inputs["representation"], np.float32)
    R = np.asarray(inputs["R"], np.float32)
    mask = np.asarray(inputs["atom_mask"], np.float32)
    W1 = np.asarray(inputs["W1"], np.float32)
    b1 = np.asarray(inputs["b1"], np.float32)
    W2 = np.asarray(inputs["W2"], np.float32)
    b2 = np.asarray(inputs["b2"], np.float32)
    Wc1 = np.asarray(inputs["Wc1"], np.float32)
    bc1 = np.asarray(inputs["bc1"], np.float32)
    Wc2 = np.asarray(inputs["Wc2"], np.float32)
    bc2 = np.asarray(inputs["bc2"], np.float32)

    # w1h[p, k*H + h] = W1[k*128+p, h]
    w1h = np.ascontiguousarray(
        W1.reshape(KD, P, H).transpose(1, 0, 2).reshape(P, KD * H))
    wc1h = np.ascontiguousarray(
        Wc1.reshape(KD, P, H).transpose(1, 0, 2).reshape(P, KD * H))
    b1t = np.ascontiguousarray(b1.reshape(HC, P).T)
    bc1t = np.ascontiguousarray(bc1.reshape(HC, P).T)
    w2t = np.ascontiguousarray(W2[:, 0].reshape(HC, P).T)
    wc2t = np.ascontiguousarray(Wc2[:, 0].reshape(HC, P).T)
    c2 = np.float32(b2[0] - LOG2 * W2.sum(dtype=np.float64))
    cq = np.float32(bc2[0] - LOG2 * Wc2.sum(dtype=np.float64))

    in_maps = []
    for c in range(NCORES):
        sl = slice(c * BL, (c + 1) * BL)
        Rb = R[sl]                                   # [BL, N, 3]
        # rcolp[p, (b*IC+ic)*3 + c] = R[b, ic*128+p, c]; rcoln negated
        rcolp = np.ascontiguousarray(
            Rb.reshape(BL, IC, P, 3).transpose(2, 0, 1, 3).reshape(P, BL * IC * 3))
        # maskc[p, b*IC+ic] = mask[b, ic*128+p]
        maskc = np.ascontiguousarray(
            mask[sl].reshape(BL, IC, P).transpose(2, 0, 1).reshape(P, BL * IC))
        # rept[b][p, k*N + n] = rep[b, n, k*128+p]
        rept = np.ascontiguousarray(
            rep[sl].reshape(BL, N, KD, P).transpose(0, 3, 2, 1).reshape(BL, P, KD * N))
        cvec = np.concatenate(
            [c2 * mask[sl].sum(axis=1, dtype=np.float32), [cq]]
        ).astype(np.float32).reshape(1, BL + 1)
        # packed smalls: rcoln | rcolp | maskc | b1t | bc1t | cvec(row 0)
        pack = np.zeros((P, BL * IC * 3 * 2 + BL * IC + 2 * HC + BL + 1),
                        np.float32)
        nc3 = BL * IC * 3
        pack[:, 0:nc3] = -rcolp
        pack[:, nc3:2 * nc3] = rcolp
        pack[:, 2 * nc3:2 * nc3 + BL * IC] = maskc
        o = 2 * nc3 + BL * IC
        pack[:, o:o + HC] = b1t
        pack[:, o + HC:o + 2 * HC] = bc1t
        pack[0, o + 2 * HC:o + 2 * HC + BL + 1] = cvec[0]
        packr = np.concatenate([w2t, wc2t], axis=1)
        in_maps.append({
            "rept": rept,
            "w1h": w1h, "wc1h": wc1h,
            "pack": pack, "packr": np.ascontiguousarray(packr),
        })
    return in_maps


def kernel(**inputs) -> np.ndarray:
    nc = _get_program()
    in_maps = _host_prep(inputs)
    res = None
    last_err = None
    for attempt in range(3):
        try:
            res = bass_utils.run_bass_kernel_spmd(
                nc, in_maps, core_ids=list(range(NCORES)))
            break
        except Exception as e:  # transient NRT_EXEC_UNIT faults have been seen
            last_err = e
            import time
            time.sleep(2.0)
            try:
                import jax
                jax.clear_backends()
            except Exception:
                pass
    if res is None:
        raise last_err
    out = np.concatenate([res.results[c]["out"][0] for c in range(NCORES)])
    return out.reshape(B, 1).astype(np.float32)


# revision 64
# speedup vs baseline: 1.0048x; 1.0048x over previous
"""Trainium2 Bass kernel for the EnergyCoulomb problem.

Reference computation (per molecule, B=32, N=512, D=1024, H=512):
  y  = sum_atoms(mask * (ssp(rep @ W1 + b1) @ W2 + b2))           atomwise MLP + pool
  q  = ssp(rep @ Wc1 + bc1) @ Wc2 + bc2                           charge net
  e  = sum_{i!=j} q_i q_j (1e-5 + |R_i - R_j|)^-2 * mask_i mask_j coulomb term
  out = y + e

Sharding: data-parallel over molecules, 4 molecules per core on 8 cores,
weights replicated.

Key design points (vs the 131.7us first-generation kernel):
  * rep is pre-transposed on the host into [128, KD*N] per molecule: the PE
    never transposes it and nothing copies transposes out of PSUM.
  * The DMA pool is effectively one serial ~330GB/s resource and every
    trigger costs ~630ns of its host sequencer's time: ALL transfers ride
    the SP ring (never the ACT ring, which would clog ACT's instruction
    queue), in consumption order, in 4KB-per-partition chunks, with all
    small inputs packed into two tensors. The z matmuls run k-major in
    half-sets so each arriving chunk immediately feeds matmuls and each
    half-set's Exps fire early, halving the PSUM-rotation stall at set
    boundaries.
  * (1e-5 + dist)^-2 is approximated by 1/d2 (max rel err ~8e-4 on the
    closest pairs, far under the 2e-2 gate): the entire sqrt chain
    (Ln, Exp, +1e-5, square) disappears. The diagonal d2 is exactly zero
    by construction; reciprocal gives inf there and affine_select
    replaces it with 0 before any consumer.
  * R rows and mask rows are built ON DEVICE (PE column->row transposes,
    DVE copy out of PSUM — GPSIMD cannot read PSUM — then Pool
    partition_broadcast) from column-spread inputs, keeping row broadcasts
    out of the serial DMA stream entirely.
  * d2 squares: x,y coords on ACT (Square with bias=-coord), z coord on
    Pool(sub)+DVE(mul) to balance engines; softplus is split so the Exps
    (which free PSUM banks the next set waits on) always sit at the head
    of the ACT queue, while the fused [128, HC*N] Ln runs one set later.
  * Pairwise chains and row builds are staggered through the schedule so
    their ACT/DVE/Pool stages fill idle windows without head-of-line
    blocking the Exps; the last chain gets dedicated d2/tmp buffers so its
    reciprocal lands before the final epilogue needs it.

ssp(x) = softplus(x) - ln2 is folded as softplus on device plus a host-side
constant shift c = b - ln2 * sum(W_layer2) applied at the pooled level.
"""

import numpy as np

import concourse.bass as bass
import concourse.bacc as bacc
import concourse.mybir as mybir
import concourse.tile as tile
from concourse import bass_utils
from concourse.masks import make_identity

LOG2 = float(np.log(2.0))

B, N, D, H = 32, 512, 1024, 512
NCORES = 8
BL = B // NCORES          # molecules per core
P = 128                   # partitions
KD = D // P               # 8 K-chunks over D
HC = H // P               # 4 h-chunks over H
IC = N // P               # 4 i-chunks over atoms

f32 = mybir.dt.float32
f32r = mybir.dt.float32r
AF = mybir.ActivationFunctionType
ALU = mybir.AluOpType
AX = mybir.AxisListType

_CACHE = {}

# Every ACT function this kernel uses (Exp, Ln, Square, Copy) lives in the
# "natural_log_exp_and_others" table set. Bacc's table chooser is
# greedy-first-match; emptying every other set (order preserved, so
# act_func_set_id indices stay valid) pins the chooser to the combined set:
# one table load for the whole kernel.
_ONE_TABLE = "natural_log_exp_and_others"


def _gat_one_table(arch):
    from concourse.hw_specs import get_activation_tables
    tabs = get_activation_tables(arch)
    assert _ONE_TABLE in tabs
    return {n: (fns if n == _ONE_TABLE else set()) for n, fns in tabs.items()}


def _build_program():
    bacc.get_activation_tables = _gat_one_table
    nc = bacc.Bacc("TRN2", target_bir_lowering=False, debug=False,
                   enable_asserts=False)

    # host-pretransposed rep: rt[b][p, k*N+n] = rep[b, n, k*128+p]
    rept_d = nc.dram_tensor("rept", [BL, P, KD * N], f32r, kind="ExternalInput").ap()
    # host-chunked weights: w1h[p, k*H+h] = W1[k*128+p, h]
    w1h_d = nc.dram_tensor("w1h", [P, KD * H], f32r, kind="ExternalInput").ap()
    wc1h_d = nc.dram_tensor("wc1h", [P, KD * H], f32r, kind="ExternalInput").ap()
    # all small inputs packed into two tensors (each DMA trigger costs
    # ~630ns of ring-sequencer time, so 8 separate smalls would delay the
    # weight stream by ~4us): packed f32 columns are
    #   rcoln[48] | rcolp[48] | maskc[16] | b1t[4] | bc1t[4] | cvec row[5]
    # where rcoln[p, (b*IC+ic)*3+c] = -R[b, ic*128+p, c] (bias/scalar
    # operands), rcolp the same un-negated (row-transpose source), and
    # maskc[p, b*IC+ic] = mask[b, ic*128+p].
    NPK = BL * IC * 3 * 2 + BL * IC + 2 * HC + (BL + 1)
    pack_d = nc.dram_tensor("pack", [P, NPK], f32, kind="ExternalInput").ap()
    packr_d = nc.dram_tensor("packr", [P, 2 * HC], f32r, kind="ExternalInput").ap()
    out_d = nc.dram_tensor("out", [1, BL], f32, kind="ExternalOutput").ap()

    with tile.TileContext(nc) as tc:
        with tc.tile_pool(name="singles", bufs=1) as singles, \
             tc.tile_pool(name="work", bufs=1) as work, \
             tc.tile_pool(name="ps", bufs=1, space="PSUM") as ps:

            ident32 = singles.tile([1, 1], f32, tag="ident32")
            nc.vector.memset(ident32, 1.0)
            ident = singles.tile([P, P], f32, tag="ident")
            make_identity(nc, ident)
            ones_col = singles.tile([P, 1], f32, tag="ones_col")
            nc.vector.memset(ones_col, 1.0)

            # ---- input streaming ----
            # Everything rides the SP-sequencer HWDGE ring in consumption
            # order: each trigger costs ~630ns of its host sequencer, so
            # putting any of these on the ACT ring would clog the ACT
            # engine's instruction queue. 4KB chunks keep the trigger rate
            # comfortably ahead of the ~330GB/s serial transfer rate while
            # still drip-feeding the k-major matmuls.
            def ring_dma(dst, src):
                nc.sync.dma_start(dst, src)

            rt_sb = []
            for b in range(BL):
                rt_sb.append(work.tile([P, KD * N], f32r, tag="rt", bufs=BL - 1,
                                       name=f"rt{b}"))
            wc1_sb = singles.tile([P, KD * H], f32r, tag="wc1h")
            w1_sb = singles.tile([P, KD * H], f32r, tag="w1h")

            pack = singles.tile([P, NPK], f32, tag="pack")
            nc.sync.dma_start(pack, pack_d)
            packr = singles.tile([P, 2 * HC], f32r, tag="packr")
            nc.sync.dma_start(packr, packr_d)
            NC3 = BL * IC * 3
            rcoln = pack[:, 0:NC3]
            rcolp = pack[:, NC3:2 * NC3]
            maskc = pack[:, 2 * NC3:2 * NC3 + BL * IC]
            _o = 2 * NC3 + BL * IC
            b1t = pack[:, _o:_o + HC]
            bc1t = pack[:, _o + HC:_o + 2 * HC]
            cvec = pack[0:1, _o + 2 * HC:_o + 2 * HC + BL + 1]
            w2t = packr[:, 0:HC]
            wc2t = packr[:, HC:2 * HC]

            # per-molecule consumption order: wc1 (mol0 charge), rt0, w1
            # (mol0 y), rt1, rt2, rt3 — in 2-k (4KB/partition) chunks
            for k in range(0, KD, 2):
                ring_dma(wc1_sb[:, k * H:(k + 2) * H], wc1h_d[:, k * H:(k + 2) * H])
                ring_dma(rt_sb[0][:, k * N:(k + 2) * N], rept_d[0][:, k * N:(k + 2) * N])
            for k in range(0, KD, 2):
                ring_dma(w1_sb[:, k * H:(k + 2) * H], w1h_d[:, k * H:(k + 2) * H])
            for b in range(1, BL):
                for k in range(0, KD, 2):
                    ring_dma(rt_sb[b][:, k * N:(k + 2) * N],
                             rept_d[b][:, k * N:(k + 2) * N])

            res = singles.tile([1, BL], f32, tag="res")

            # ---- device-side row builds (replaces row-broadcast DMAs) ----
            xjb_tiles = {}
            mrows = {}

            def build_xjb(b):
                # GPSIMD cannot read PSUM: transpose -> DVE copy to an SBUF
                # row -> Pool broadcast
                xjb = work.tile([P, 3, N], f32, tag="xjb", bufs=2)
                xrow = work.tile([1, 3, N], f32, tag="xrow", bufs=1)
                for c in range(3):
                    xp = ps.tile([1, N], f32, tag="xrow_ps", bufs=1)
                    for ic in range(IC):
                        nc.tensor.transpose(
                            xp[0:1, ic * P:(ic + 1) * P],
                            rcolp[:, (b * IC + ic) * 3 + c:(b * IC + ic) * 3 + c + 1],
                            ident)
                    nc.vector.tensor_copy(xrow[0:1, c, :], xp)
                nc.gpsimd.partition_broadcast(xjb, xrow)
                xjb_tiles[b] = xjb

            def build_mrow(b):
                mp = ps.tile([1, N], f32, tag="xrow_ps", bufs=1)
                for ic in range(IC):
                    nc.tensor.transpose(
                        mp[0:1, ic * P:(ic + 1) * P],
                        maskc[:, b * IC + ic:b * IC + ic + 1],
                        ident)
                m = singles.tile([1, N], f32, tag=f"mrow_{b}")
                nc.vector.tensor_copy(m, mp)
                mrows[b] = m

            # ---- pairwise chain: rb[p, ic, j] = 1/d2_(128ic+p),j  (0 diag) --
            # Split into an ACT-square part and a DVE/Pool part so each can
            # be placed independently in its engine's in-order queue.
            def chain_sq(b):
                xjb = xjb_tiles[b]
                tag_sfx = "3" if b == 3 else ""
                d2b = work.tile([P, IC, N], f32, tag="d2b" + tag_sfx, bufs=1)
                tmpb = work.tile([P, IC, N], f32, tag="tmpb" + tag_sfx, bufs=1)
                for ic in range(IC):
                    col = (b * IC + ic) * 3
                    nc.scalar.activation(d2b[:, ic, :], xjb[:, 0, :], AF.Square,
                                         bias=rcoln[:, col + 0:col + 1])
                    nc.scalar.activation(tmpb[:, ic, :], xjb[:, 1, :], AF.Square,
                                         bias=rcoln[:, col + 1:col + 2])
                return d2b, tmpb

            def chain_rest(b, d2b, tmpb, dve_subs=False):
                xjb = xjb_tiles[b]
                nc.vector.tensor_tensor(d2b, d2b, tmpb, op=ALU.add)
                # z coord: subtract on Pool (DVE for the last chain, whose
                # latency is on the final epilogue's critical path), square +
                # accumulate on DVE
                sub_eng = nc.vector if dve_subs else nc.gpsimd
                for ic in range(IC):
                    col = (b * IC + ic) * 3
                    sub_eng.tensor_scalar(tmpb[:, ic, :], xjb[:, 2, :],
                                          rcoln[:, col + 2:col + 3], None,
                                          op0=ALU.add)
                nc.vector.tensor_mul(tmpb, tmpb, tmpb)
                nc.vector.tensor_tensor(d2b, d2b, tmpb, op=ALU.add)
                nc.vector.reciprocal(d2b, d2b)
                rb = work.tile([P, IC, N], f32r, tag="rb", bufs=2)
                nc.gpsimd.affine_select(
                    out=rb, in_=d2b, compare_op=ALU.not_equal, fill=0.0,
                    base=0, pattern=[[P, IC], [-1, N]], channel_multiplier=1)
                return rb

            rb_tiles = {}
            h_tiles = {}
            chain_mid = {}

            # ---- per-molecule MLP set, two phases ----
            # z-phase: k-major z matmuls, then the Exps IMMEDIATELY (they
            # free the PSUM slots the next set's matmuls are waiting on).
            # ln-phase: the (fused, bias-free) Ln runs later, in the next
            # set's z-window, so it never blocks exps in the in-order ACT
            # queue.
            ez_tiles = {}

            def emit_mlp_z(b, wset):
                # half-sets (hc 0,1 then hc 2,3): the first half's exps fire
                # ~3.4us before the set's end, so the next set's matmuls wait
                # only on the second half's exps (halved boundary stall).
                w_sb, bias = (wc1_sb, bc1t) if wset == "q" else (w1_sb, b1t)
                zts = [ps.tile([P, N], f32, tag="z", bufs=5,
                               name=f"z_{b}_{wset}_{hc}") for hc in range(HC)]
                ez = work.tile([P, HC, N], f32, tag="ez", bufs=2)
                for half in range(2):
                    hcs = (0, 1) if half == 0 else (2, 3)
                    for k in range(KD):
                        for hc in hcs:
                            nc.tensor.matmul(
                                zts[hc],
                                lhsT=w_sb[:, k * H + hc * P:k * H + (hc + 1) * P],
                                rhs=rt_sb[b][:, k * N:(k + 1) * N],
                                start=(k == 0), stop=(k == KD - 1))
                    for hc in hcs:
                        nc.scalar.activation(ez[:, hc, :], zts[hc], AF.Exp,
                                             bias=bias[:, hc:hc + 1])
                ez_tiles[(b, wset)] = ez

            def emit_mlp_ln(b, wset, halves=False):
                ez = ez_tiles.pop((b, wset))
                h = work.tile([P, HC, N], f32r, tag=f"h_{wset}", bufs=2)
                if halves:
                    nc.scalar.activation(h[:, 0:2, :], ez[:, 0:2, :], AF.Ln,
                                         bias=ones_col[:, 0:1])
                    nc.scalar.activation(h[:, 2:4, :], ez[:, 2:4, :], AF.Ln,
                                         bias=ones_col[:, 0:1])
                else:
                    nc.scalar.activation(h, ez, AF.Ln, bias=ones_col[:, 0:1])
                h_tiles[(b, wset)] = h

            # ---- per-molecule epilogue (charge front / finish split) ----
            epi_front = {}

            def emit_epi_front(b):
                """q row + q columns — needs only ln(b, q) and the mask."""
                hq = h_tiles.pop((b, "q"))
                q_ps = ps.tile([1, N], f32, tag="row_ps", bufs=2)
                for hc in range(HC):
                    nc.tensor.matmul(q_ps,
                                     lhsT=wc2t[:, hc:hc + 1],
                                     rhs=hq[:, hc, :],
                                     start=(hc == 0), stop=(hc == HC - 1))
                qrow = work.tile([1, N], f32, tag="qrow", bufs=1)
                nc.vector.tensor_scalar(qrow, q_ps, cvec[0:1, BL:BL + 1], None,
                                        op0=ALU.add)
                nc.vector.tensor_mul(qrow, qrow, mrows[b])

                qc_ps = ps.tile([P, IC], f32, tag="row_ps", bufs=2)
                for ic in range(IC):
                    nc.tensor.transpose(qc_ps[:, ic:ic + 1],
                                        qrow[:, ic * P:(ic + 1) * P],
                                        ident32[0:1, 0:1])
                qc = work.tile([P, IC], f32r, tag="qc", bufs=2)
                nc.vector.tensor_copy(qc, qc_ps)
                epi_front[b] = (qrow, qc)

            def emit_epilogue(b):
                if b not in epi_front:
                    emit_epi_front(b)
                qrow, qc = epi_front.pop(b)
                mrow = mrows[b]
                rb = rb_tiles.pop(b)
                t_ps = ps.tile([1, N], f32, tag="row_ps", bufs=2)
                for ic in range(IC):
                    nc.tensor.matmul(t_ps,
                                     lhsT=qc[:, ic:ic + 1],
                                     rhs=rb[:, ic, :],
                                     start=(ic == 0), stop=(ic == IC - 1))
                scr_e = work.tile([1, N], f32, tag="scr", bufs=1)
                nc.vector.tensor_mul(scr_e, t_ps, qrow)
                e_sb = work.tile([1, 1], f32, tag="e_sb", bufs=2)
                nc.vector.reduce_sum(e_sb, scr_e, axis=AX.X)

                h1 = h_tiles.pop((b, "y"))
                yi_ps = ps.tile([1, N], f32, tag="row_ps", bufs=2)
                for hc in range(HC):
                    nc.tensor.matmul(yi_ps,
                                     lhsT=w2t[:, hc:hc + 1],
                                     rhs=h1[:, hc, :],
                                     start=(hc == 0), stop=(hc == HC - 1))
                scr_y = work.tile([1, N], f32, tag="scr", bufs=1)
                nc.vector.tensor_mul(scr_y, yi_ps, mrow)
                ysum = work.tile([1, 1], f32, tag="ysum", bufs=2)
                nc.vector.reduce_sum(ysum, scr_y, axis=AX.X)
                y_sb = work.tile([1, 1], f32, tag="y_sb", bufs=2)
                nc.vector.tensor_add(y_sb, ysum, cvec[0:1, b:b + 1])
                nc.vector.tensor_add(res[:, b:b + 1], y_sb, e_sb)

            # ---- schedule ----
            # Row builds + all four pairwise chains go first: their ACT
            # squares fill the DMA-drip phase where ACT is otherwise idle,
            # and every rb is ready long before its epilogue.
            build_xjb(0)
            chain_mid[0] = chain_sq(0)
            emit_mlp_z(0, "q")
            build_xjb(1)                      # PE transposes fill the boundary
            rb_tiles[0] = chain_rest(0, *chain_mid.pop(0))
            emit_mlp_z(0, "y")
            chain_mid[1] = chain_sq(1)
            emit_mlp_ln(0, "q")
            emit_mlp_z(1, "q")
            build_xjb(2)
            rb_tiles[1] = chain_rest(1, *chain_mid.pop(1))
            build_mrow(0)
            emit_epi_front(0)
            emit_mlp_ln(0, "y")
            emit_epilogue(0)
            emit_mlp_z(1, "y")
            chain_mid[2] = chain_sq(2)
            emit_mlp_ln(1, "q")
            emit_mlp_z(2, "q")
            build_xjb(3)
            chain_mid[3] = chain_sq(3)
            rb_tiles[2] = chain_rest(2, *chain_mid.pop(2))
            rb_tiles[3] = chain_rest(3, *chain_mid.pop(3))
            build_mrow(1)
            emit_epi_front(1)
            emit_mlp_ln(1, "y")
            emit_epilogue(1)
            emit_mlp_z(2, "y")
            emit_mlp_ln(2, "q")
            build_mrow(2)
            emit_epi_front(2)
            emit_mlp_z(3, "q")
            emit_mlp_ln(2, "y")
            emit_epilogue(2)
            emit_mlp_ln(3, "q")
            build_mrow(3)
            emit_epi_front(3)
            emit_mlp_z(3, "y")
            emit_mlp_ln(3, "y", halves=True)
            emit_epilogue(3)

            nc.sync.dma_start(out_d, res)

    nc.compile()
    return nc


def _get_program():
    if "nc" not in _CACHE:
        _CACHE["nc"] = _build_program()
    return _CACHE["nc"]


def _host_prep(inputs):
    """Build per-core in_maps from full inputs."""
    rep = np.asarray(inputs["representation"], np.float32)
    R = np.asarray(inputs["R"], np.float32)
    mask = np.asarray(inputs["atom_mask"], np.float32)
    W1 = np.asarray(inputs["W1"], np.float32)
    b1 = np.asarray(inputs["b1"], np.float32)
    W2 = np.asarray(inputs["W2"], np.float32)
    b2 = np.asarray(inputs["b2"], np.float32)
    Wc1 = np.asarray(inputs["Wc1"], np.float32)
    bc1 = np.asarray(inputs["bc1"], np.float32)
    Wc2 = np.asarray(inputs["Wc2"], np.float32)
    bc2 = np.asarray(inputs["bc2"], np.float32)

    # w1h[p, k*H + h] = W1[k*128+p, h]
    w1h = np.ascontiguousarray(
        W1.reshape(KD, P, H).transpose(1, 0, 2).reshape(P, KD * H))
    wc1h = np.ascontiguousarray(
        Wc1.reshape(KD, P, H).transpose(1, 0, 2).reshape(P, KD * H))
    b1t = np.ascontiguousarray(b1.reshape(HC, P).T)
    bc1t = np.ascontiguousarray(bc1.reshape(HC, P).T)
    w2t = np.ascontiguousarray(W2[:, 0].reshape(HC, P).T)
    wc2t = np.ascontiguousarray(Wc2[:, 0].reshape(HC, P).T)
    c2 = np.float32(b2[0] - LOG2 * W2.sum(dtype=np.float64))
    cq = np.float32(bc2[0] - LOG2 * Wc2.sum(dtype=np.float64))

    in_maps = []
    for c in range(NCORES):
        sl = slice(c * BL, (c + 1) * BL)
        Rb = R[sl]                                   # [BL, N, 3]
        # rcolp[p, (b*IC+ic)*3 + c] = R[b, ic*128+p, c]; rcoln negated
        rcolp = np.ascontiguousarray(
            Rb.reshape(BL, IC, P, 3).transpose(2, 0, 1, 3).reshape(P, BL * IC * 3))
        # maskc[p, b*IC+ic] = mask[b, ic*128+p]
        maskc = np.ascontiguousarray(
            mask[sl].reshape(BL, IC, P).transpose(2, 0, 1).reshape(P, BL * IC))
        # rept[b][p, k*N + n] = rep[b, n, k*128+p]
        rept = np.ascontiguousarray(
            rep[sl].reshape(BL, N, KD, P).transpose(0, 3, 2, 1).reshape(BL, P, KD * N))
        cvec = np.concatenate(
            [c2 * mask[sl].sum(axis=1, dtype=np.float32), [cq]]
        ).astype(np.float32).reshape(1, BL + 1)
        # packed smalls: rcoln | rcolp | maskc | b1t | bc1t | cvec(row 0)
        pack = np.zeros((P, BL * IC * 3 * 2 + BL * IC + 2 * HC + BL + 1),
                        np.float32)
        nc3 = BL * IC * 3
        pack[:, 0:nc3] = -rcolp
        pack[:, nc3:2 * nc3] = rcolp
        pack[:, 2 * nc3:2 * nc3 + BL * IC] = maskc
        o = 2 * nc3 + BL * IC
        pack[:, o:o + HC] = b1t
        pack[:, o + HC:o + 2 * HC] = bc1t
        pack[0, o + 2 * HC:o + 2 * HC + BL + 1] = cvec[0]
        packr = np.concatenate([w2t, wc2t], axis=1)
        in_maps.append({
            "rept": rept,
            "w1h": w1h, "wc1h": wc1h,
            "pack": pack, "packr": np.ascontiguousarray(packr),
        })
    return in_maps


def kernel(**inputs) -> np.ndarray:
    nc = _get_program()
    in_maps = _host_prep(inputs)
    res = None
    last_err = None
    for attempt in range(3):
        try:
            res = bass_utils.run_bass_kernel_spmd(
                nc, in_maps, core_ids=list(range(NCORES)))
            break
        except Exception as e:  # transient NRT_EXEC_UNIT faults have been seen
            last_err = e
            import time
            time.sleep(2.0)
            try:
                import jax
                jax.clear_backends()
            except Exception:
                pass
    if res is None:
        raise last_err
    out = np.concatenate([res.results[c]["out"][0] for c in range(NCORES)])
    return out.reshape(B, 1).astype(np.float32)


# revision 70
# speedup vs baseline: 1.0413x; 1.0363x over previous
"""Trainium2 Bass kernel for the EnergyCoulomb problem.

Reference computation (per molecule, B=32, N=512, D=1024, H=512):
  y  = sum_atoms(mask * (ssp(rep @ W1 + b1) @ W2 + b2))           atomwise MLP + pool
  q  = ssp(rep @ Wc1 + bc1) @ Wc2 + bc2                           charge net
  e  = sum_{i!=j} q_i q_j (1e-5 + |R_i - R_j|)^-2 * mask_i mask_j coulomb term
  out = y + e

Sharding: data-parallel over molecules, 4 molecules per core on 8 cores,
weights replicated.

Key design points (vs the 131.7us first-generation kernel):
  * rep is pre-transposed on the host into [128, KD*N] per molecule: the PE
    never transposes it and nothing copies transposes out of PSUM.
  * The DMA pool is effectively one serial ~330GB/s resource and every
    trigger costs ~630ns of its host sequencer's time: ALL transfers ride
    the SP ring (never the ACT ring, which would clog ACT's instruction
    queue), in consumption order, in 4KB-per-partition chunks, with all
    small inputs packed into two tensors. The z matmuls run k-major in
    half-sets so each arriving chunk immediately feeds matmuls and each
    half-set's Exps fire early, halving the PSUM-rotation stall at set
    boundaries.
  * (1e-5 + dist)^-2 is approximated by 1/d2 (max rel err ~8e-4 on the
    closest pairs, far under the 2e-2 gate): the entire sqrt chain
    (Ln, Exp, +1e-5, square) disappears. The diagonal d2 is exactly zero
    by construction; reciprocal gives inf there and affine_select
    replaces it with 0 before any consumer.
  * R rows and mask rows are built ON DEVICE (PE column->row transposes,
    DVE copy out of PSUM — GPSIMD cannot read PSUM — then Pool
    partition_broadcast) from column-spread inputs, keeping row broadcasts
    out of the serial DMA stream entirely.
  * d2 squares: x,y coords on ACT (Square with bias=-coord), z coord on
    Pool(sub)+DVE(mul) to balance engines; softplus is split so the Exps
    (which free PSUM banks the next set waits on) always sit at the head
    of the ACT queue, while the fused [128, HC*N] Ln runs one set later.
  * Pairwise chains and row builds are staggered through the schedule so
    their ACT/DVE/Pool stages fill idle windows without head-of-line
    blocking the Exps; the last chain gets dedicated d2/tmp buffers so its
    reciprocal lands before the final epilogue needs it.

ssp(x) = softplus(x) - ln2 is folded as softplus on device plus a host-side
constant shift c = b - ln2 * sum(W_layer2) applied at the pooled level.
"""

import numpy as np

import concourse.bass as bass
import concourse.bacc as bacc
import concourse.mybir as mybir
import concourse.tile as tile
from concourse import bass_utils
from concourse.masks import make_identity

LOG2 = float(np.log(2.0))

B, N, D, H = 32, 512, 1024, 512
NCORES = 8
BL = B // NCORES          # molecules per core
P = 128                   # partitions
KD = D // P               # 8 K-chunks over D
HC = H // P               # 4 h-chunks over H
IC = N // P               # 4 i-chunks over atoms

f32 = mybir.dt.float32
f32r = mybir.dt.float32r
AF = mybir.ActivationFunctionType
ALU = mybir.AluOpType
AX = mybir.AxisListType

_CACHE = {}

# Every ACT function this kernel uses (Exp, Ln, Square, Copy) lives in the
# "natural_log_exp_and_others" table set. Bacc's table chooser is
# greedy-first-match; emptying every other set (order preserved, so
# act_func_set_id indices stay valid) pins the chooser to the combined set:
# one table load for the whole kernel.
_ONE_TABLE = "natural_log_exp_and_others"


def _gat_one_table(arch):
    from concourse.hw_specs import get_activation_tables
    tabs = get_activation_tables(arch)
    assert _ONE_TABLE in tabs
    return {n: (fns if n == _ONE_TABLE else set()) for n, fns in tabs.items()}


def _build_program():
    bacc.get_activation_tables = _gat_one_table
    nc = bacc.Bacc("TRN2", target_bir_lowering=False, debug=False,
                   enable_asserts=False)

    # host-pretransposed rep: rt[b][p, k*N+n] = rep[b, n, k*128+p]
    rept_d = nc.dram_tensor("rept", [BL, P, KD * N], f32r, kind="ExternalInput").ap()
    # host-chunked weights: w1h[p, k*H+h] = W1[k*128+p, h]
    w1h_d = nc.dram_tensor("w1h", [P, KD * H], f32r, kind="ExternalInput").ap()
    wc1h_d = nc.dram_tensor("wc1h", [P, KD * H], f32r, kind="ExternalInput").ap()
    # all small inputs packed into two tensors (each DMA trigger costs
    # ~630ns of ring-sequencer time, so 8 separate smalls would delay the
    # weight stream by ~4us): packed f32 columns are
    #   rcoln[48] | rcolp[48] | maskc[16] | b1t[4] | bc1t[4] | cvec row[5]
    # where rcoln[p, (b*IC+ic)*3+c] = -R[b, ic*128+p, c] (bias/scalar
    # operands), rcolp the same un-negated (row-transpose source), and
    # maskc[p, b*IC+ic] = mask[b, ic*128+p].
    NPK = BL * IC * 3 * 2 + BL * IC + 2 * HC + (BL + 1)
    pack_d = nc.dram_tensor("pack", [P, NPK], f32, kind="ExternalInput").ap()
    packr_d = nc.dram_tensor("packr", [P, 2 * HC], f32r, kind="ExternalInput").ap()
    out_d = nc.dram_tensor("out", [1, BL], f32, kind="ExternalOutput").ap()

    with tile.TileContext(nc) as tc:
        with tc.tile_pool(name="singles", bufs=1) as singles, \
             tc.tile_pool(name="work", bufs=1) as work, \
             tc.tile_pool(name="ps", bufs=1, space="PSUM") as ps:

            ident32 = singles.tile([1, 1], f32, tag="ident32")
            nc.vector.memset(ident32, 1.0)
            ident = singles.tile([P, P], f32, tag="ident")
            make_identity(nc, ident)
            ones_col = singles.tile([P, 1], f32, tag="ones_col")
            nc.vector.memset(ones_col, 1.0)

            # ---- input streaming ----
            # Everything rides the SP-sequencer HWDGE ring in consumption
            # order: each trigger costs ~630ns of its host sequencer, so
            # putting any of these on the ACT ring would clog the ACT
            # engine's instruction queue. 4KB chunks keep the trigger rate
            # comfortably ahead of the ~330GB/s serial transfer rate while
            # still drip-feeding the k-major matmuls.
            def ring_dma(dst, src):
                nc.sync.dma_start(dst, src)

            rt_sb = []
            for b in range(BL):
                rt_sb.append(work.tile([P, KD * N], f32r, tag="rt", bufs=BL - 1,
                                       name=f"rt{b}"))
            wc1_sb = singles.tile([P, KD * H], f32r, tag="wc1h")
            w1_sb = singles.tile([P, KD * H], f32r, tag="w1h")

            pack = singles.tile([P, NPK], f32, tag="pack")
            nc.sync.dma_start(pack, pack_d)
            packr = singles.tile([P, 2 * HC], f32r, tag="packr")
            nc.sync.dma_start(packr, packr_d)
            NC3 = BL * IC * 3
            rcoln = pack[:, 0:NC3]
            rcolp = pack[:, NC3:2 * NC3]
            maskc = pack[:, 2 * NC3:2 * NC3 + BL * IC]
            _o = 2 * NC3 + BL * IC
            b1t = pack[:, _o:_o + HC]
            bc1t = pack[:, _o + HC:_o + 2 * HC]
            cvec = pack[0:1, _o + 2 * HC:_o + 2 * HC + BL + 1]
            w2t = packr[:, 0:HC]
            wc2t = packr[:, HC:2 * HC]

            # per-molecule consumption order: wc1 (mol0 charge), rt0, w1
            # (mol0 y), rt1, rt2, rt3 — in 2-k (4KB/partition) chunks
            for k in range(0, KD, 2):
                ring_dma(wc1_sb[:, k * H:(k + 2) * H], wc1h_d[:, k * H:(k + 2) * H])
                ring_dma(rt_sb[0][:, k * N:(k + 2) * N], rept_d[0][:, k * N:(k + 2) * N])
            for k in range(0, KD, 2):
                ring_dma(w1_sb[:, k * H:(k + 2) * H], w1h_d[:, k * H:(k + 2) * H])
            for b in range(1, BL):
                for k in range(0, KD, 2):
                    ring_dma(rt_sb[b][:, k * N:(k + 2) * N],
                             rept_d[b][:, k * N:(k + 2) * N])

            res = singles.tile([1, BL], f32, tag="res")

            # ---- device-side row builds (replaces row-broadcast DMAs) ----
            xjb_tiles = {}
            mrows = {}

            def build_xjb(b):
                # GPSIMD cannot read PSUM: transpose -> DVE copy to an SBUF
                # row -> Pool broadcast
                xjb = work.tile([P, 3, N], f32, tag="xjb", bufs=2)
                xrow = work.tile([1, 3, N], f32, tag="xrow", bufs=1)
                for c in range(3):
                    xp = ps.tile([1, N], f32, tag="xrow_ps", bufs=1)
                    for ic in range(IC):
                        nc.tensor.transpose(
                            xp[0:1, ic * P:(ic + 1) * P],
                            rcolp[:, (b * IC + ic) * 3 + c:(b * IC + ic) * 3 + c + 1],
                            ident)
                    nc.vector.tensor_copy(xrow[0:1, c, :], xp)
                nc.gpsimd.partition_broadcast(xjb, xrow)
                xjb_tiles[b] = xjb

            def build_mrow(b):
                mp = ps.tile([1, N], f32, tag="xrow_ps", bufs=1)
                for ic in range(IC):
                    nc.tensor.transpose(
                        mp[0:1, ic * P:(ic + 1) * P],
                        maskc[:, b * IC + ic:b * IC + ic + 1],
                        ident)
                m = singles.tile([1, N], f32, tag=f"mrow_{b}")
                nc.vector.tensor_copy(m, mp)
                mrows[b] = m

            # ---- pairwise chain: rb[p, ic, j] = 1/d2_(128ic+p),j  (0 diag) --
            # Split into an ACT-square part and a DVE/Pool part so each can
            # be placed independently in its engine's in-order queue.
            def chain_sq(b):
                # two buffer pairs alternating by parity: chain k only
                # serializes against chain k-2, so chains 2/3 can run a full
                # stage earlier and their reciprocals clear the tail
                xjb = xjb_tiles[b]
                tag_sfx = "AB"[b % 2]
                d2b = work.tile([P, IC, N], f32, tag="d2b" + tag_sfx, bufs=1)
                tmpb = work.tile([P, IC, N], f32, tag="tmpb" + tag_sfx, bufs=1)
                for ic in range(IC):
                    col = (b * IC + ic) * 3
                    nc.scalar.activation(d2b[:, ic, :], xjb[:, 0, :], AF.Square,
                                         bias=rcoln[:, col + 0:col + 1])
                    nc.scalar.activation(tmpb[:, ic, :], xjb[:, 1, :], AF.Square,
                                         bias=rcoln[:, col + 1:col + 2])
                return d2b, tmpb

            def chain_rest(b, d2b, tmpb, dve_subs=False):
                xjb = xjb_tiles[b]
                nc.vector.tensor_tensor(d2b, d2b, tmpb, op=ALU.add)
                # z coord: subtract on Pool (DVE for the last chain, whose
                # latency is on the final epilogue's critical path), square +
                # accumulate on DVE
                sub_eng = nc.vector if dve_subs else nc.gpsimd
                for ic in range(IC):
                    col = (b * IC + ic) * 3
                    sub_eng.tensor_scalar(tmpb[:, ic, :], xjb[:, 2, :],
                                          rcoln[:, col + 2:col + 3], None,
                                          op0=ALU.add)
                nc.vector.tensor_mul(tmpb, tmpb, tmpb)
                nc.vector.tensor_tensor(d2b, d2b, tmpb, op=ALU.add)
                nc.vector.reciprocal(d2b, d2b)
                rb = work.tile([P, IC, N], f32r, tag="rb", bufs=2)
                nc.gpsimd.affine_select(
                    out=rb, in_=d2b, compare_op=ALU.not_equal, fill=0.0,
                    base=0, pattern=[[P, IC], [-1, N]], channel_multiplier=1)
                return rb

            rb_tiles = {}
            h_tiles = {}
            chain_mid = {}

            # ---- per-molecule MLP set, two phases ----
            # z-phase: k-major z matmuls, then the Exps IMMEDIATELY (they
            # free the PSUM slots the next set's matmuls are waiting on).
            # ln-phase: the (fused, bias-free) Ln runs later, in the next
            # set's z-window, so it never blocks exps in the in-order ACT
            # queue.
            ez_tiles = {}

            def emit_mlp_z(b, wset):
                # half-sets (hc 0,1 then hc 2,3): the first half's exps fire
                # ~3.4us before the set's end, so the next set's matmuls wait
                # only on the second half's exps (halved boundary stall).
                w_sb, bias = (wc1_sb, bc1t) if wset == "q" else (w1_sb, b1t)
                zts = [ps.tile([P, N], f32, tag="z", bufs=5,
                               name=f"z_{b}_{wset}_{hc}") for hc in range(HC)]
                ez = work.tile([P, HC, N], f32, tag="ez", bufs=2)
                for half in range(2):
                    hcs = (0, 1) if half == 0 else (2, 3)
                    for k in range(KD):
                        for hc in hcs:
                            nc.tensor.matmul(
                                zts[hc],
                                lhsT=w_sb[:, k * H + hc * P:k * H + (hc + 1) * P],
                                rhs=rt_sb[b][:, k * N:(k + 1) * N],
                                start=(k == 0), stop=(k == KD - 1))
                    for hc in hcs:
                        nc.scalar.activation(ez[:, hc, :], zts[hc], AF.Exp,
                                             bias=bias[:, hc:hc + 1])
                ez_tiles[(b, wset)] = ez

            def emit_mlp_ln(b, wset, halves=False):
                ez = ez_tiles.pop((b, wset))
                h = work.tile([P, HC, N], f32r, tag=f"h_{wset}", bufs=2)
                if halves:
                    nc.scalar.activation(h[:, 0:2, :], ez[:, 0:2, :], AF.Ln,
                                         bias=ones_col[:, 0:1])
                    nc.scalar.activation(h[:, 2:4, :], ez[:, 2:4, :], AF.Ln,
                                         bias=ones_col[:, 0:1])
                else:
                    nc.scalar.activation(h, ez, AF.Ln, bias=ones_col[:, 0:1])
                h_tiles[(b, wset)] = h

            # ---- per-molecule epilogue (charge front / finish split) ----
            epi_front = {}

            def emit_epi_front(b):
                """q row + q columns — needs only ln(b, q) and the mask."""
                hq = h_tiles.pop((b, "q"))
                q_ps = ps.tile([1, N], f32, tag="row_ps", bufs=2)
                for hc in range(HC):
                    nc.tensor.matmul(q_ps,
                                     lhsT=wc2t[:, hc:hc + 1],
                                     rhs=hq[:, hc, :],
                                     start=(hc == 0), stop=(hc == HC - 1))
                qrow = work.tile([1, N], f32, tag="qrow", bufs=1)
                nc.vector.tensor_scalar(qrow, q_ps, cvec[0:1, BL:BL + 1], None,
                                        op0=ALU.add)
                nc.vector.tensor_mul(qrow, qrow, mrows[b])

                qc_ps = ps.tile([P, IC], f32, tag="row_ps", bufs=2)
                for ic in range(IC):
                    nc.tensor.transpose(qc_ps[:, ic:ic + 1],
                                        qrow[:, ic * P:(ic + 1) * P],
                                        ident32[0:1, 0:1])
                qc = work.tile([P, IC], f32r, tag="qc", bufs=2)
                nc.vector.tensor_copy(qc, qc_ps)
                epi_front[b] = (qrow, qc)

            epi_mid = {}

            def emit_epi_mid(b):
                """coulomb t-matvec + e reduction — needs rb and the front."""
                qrow, qc = epi_front.pop(b)
                rb = rb_tiles.pop(b)
                t_ps = ps.tile([1, N], f32, tag="row_ps", bufs=2)
                for ic in range(IC):
                    nc.tensor.matmul(t_ps,
                                     lhsT=qc[:, ic:ic + 1],
                                     rhs=rb[:, ic, :],
                                     start=(ic == 0), stop=(ic == IC - 1))
                scr_e = work.tile([1, N], f32, tag="scr", bufs=1)
                nc.vector.tensor_mul(scr_e, t_ps, qrow)
                e_sb = work.tile([1, 1], f32, tag="e_sb", bufs=2)
                nc.vector.reduce_sum(e_sb, scr_e, axis=AX.X)
                epi_mid[b] = e_sb

            def emit_epilogue(b):
                if b not in epi_front and b not in epi_mid:
                    emit_epi_front(b)
                if b not in epi_mid:
                    emit_epi_mid(b)
                e_sb = epi_mid.pop(b)
                h1 = h_tiles.pop((b, "y"))
                yi_ps = ps.tile([1, N], f32, tag="row_ps", bufs=2)
                for hc in range(HC):
                    nc.tensor.matmul(yi_ps,
                                     lhsT=w2t[:, hc:hc + 1],
                                     rhs=h1[:, hc, :],
                                     start=(hc == 0), stop=(hc == HC - 1))
                scr_y = work.tile([1, N], f32, tag="scr", bufs=1)
                nc.vector.tensor_mul(scr_y, yi_ps, mrows[b])
                ysum = work.tile([1, 1], f32, tag="ysum", bufs=2)
                nc.vector.reduce_sum(ysum, scr_y, axis=AX.X)
                # res[b] = (ysum + cvec_b) + e_sb in a single two-op pass
                nc.vector.tensor_scalar(res[:, b:b + 1], ysum,
                                        cvec[0:1, b:b + 1], e_sb,
                                        op0=ALU.add, op1=ALU.add)

            # ---- schedule ----
            # Row builds + all four pairwise chains go first: their ACT
            # squares fill the DMA-drip phase where ACT is otherwise idle,
            # and every rb is ready long before its epilogue.
            build_xjb(0)
            chain_mid[0] = chain_sq(0)
            emit_mlp_z(0, "q")
            build_xjb(1)                      # PE transposes fill the boundary
            rb_tiles[0] = chain_rest(0, *chain_mid.pop(0))
            emit_mlp_z(0, "y")
            chain_mid[1] = chain_sq(1)
            emit_mlp_ln(0, "q")
            build_mrow(0)
            emit_epi_front(0)
            emit_mlp_z(1, "q")
            build_xjb(2)
            rb_tiles[1] = chain_rest(1, *chain_mid.pop(1))
            chain_mid[2] = chain_sq(2)
            rb_tiles[2] = chain_rest(2, *chain_mid.pop(2))
            emit_epi_mid(0)
            emit_mlp_ln(0, "y")
            emit_epilogue(0)
            emit_mlp_z(1, "y")
            build_xjb(3)
            chain_mid[3] = chain_sq(3)
            rb_tiles[3] = chain_rest(3, *chain_mid.pop(3))
            emit_mlp_ln(1, "q")
            build_mrow(1)
            emit_epi_front(1)
            emit_epi_mid(1)
            emit_mlp_z(2, "q")
            emit_mlp_ln(1, "y")
            emit_epilogue(1)
            emit_mlp_z(2, "y")
            emit_mlp_ln(2, "q")
            build_mrow(2)
            emit_epi_front(2)
            emit_epi_mid(2)
            emit_mlp_z(3, "q")
            emit_mlp_ln(2, "y")
            emit_epilogue(2)
            emit_mlp_ln(3, "q")
            build_mrow(3)
            emit_epi_front(3)
            emit_mlp_z(3, "y")
            emit_mlp_ln(3, "y", halves=True)
            emit_epilogue(3)

            nc.sync.dma_start(out_d, res)

    nc.compile()
    return nc


def _get_program():
    if "nc" not in _CACHE:
        _CACHE["nc"] = _build_program()
    return _CACHE["nc"]


def _host_prep(inputs):
    """Build per-core in_maps from full inputs."""
    rep = np.asarray(inputs["representation"], np.float32)
    R = np.asarray(inputs["R"], np.float32)
    mask = np.asarray(inputs["atom_mask"], np.float32)
    W1 = np.asarray(inputs["W1"], np.float32)
    b1 = np.asarray(inputs["b1"], np.float32)
    W2 = np.asarray(inputs["W2"], np.float32)
    b2 = np.asarray(inputs["b2"], np.float32)
    Wc1 = np.asarray(inputs["Wc1"], np.float32)
    bc1 = np.asarray(inputs["bc1"], np.float32)
    Wc2 = np.asarray(inputs["Wc2"], np.float32)
    bc2 = np.asarray(inputs["bc2"], np.float32)

    # w1h[p, k*H + h] = W1[k*128+p, h]
    w1h = np.ascontiguousarray(
        W1.reshape(KD, P, H).transpose(1, 0, 2).reshape(P, KD * H))
    wc1h = np.ascontiguousarray(
        Wc1.reshape(KD, P, H).transpose(1, 0, 2).reshape(P, KD * H))
    b1t = np.ascontiguousarray(b1.reshape(HC, P).T)
    bc1t = np.ascontiguousarray(bc1.reshape(HC, P).T)
    w2t = np.ascontiguousarray(W2[:, 0].reshape(HC, P).T)
    wc2t = np.ascontiguousarray(Wc2[:, 0].reshape(HC, P).T)
    c2 = np.float32(b2[0] - LOG2 * W2.sum(dtype=np.float64))
    cq = np.float32(bc2[0] - LOG2 * Wc2.sum(dtype=np.float64))

    in_maps = []
    for c in range(NCORES):
        sl = slice(c * BL, (c + 1) * BL)
        Rb = R[sl]                                   # [BL, N, 3]
        # rcolp[p, (b*IC+ic)*3 + c] = R[b, ic*128+p, c]; rcoln negated
        rcolp = np.ascontiguousarray(
            Rb.reshape(BL, IC, P, 3).transpose(2, 0, 1, 3).reshape(P, BL * IC * 3))
        # maskc[p, b*IC+ic] = mask[b, ic*128+p]
        maskc = np.ascontiguousarray(
            mask[sl].reshape(BL, IC, P).transpose(2, 0, 1).reshape(P, BL * IC))
        # rept[b][p, k*N + n] = rep[b, n, k*128+p]
        rept = np.ascontiguousarray(
            rep[sl].reshape(BL, N, KD, P).transpose(0, 3, 2, 1).reshape(BL, P, KD * N))
        cvec = np.concatenate(
            [c2 * mask[sl].sum(axis=1, dtype=np.float32), [cq]]
        ).astype(np.float32).reshape(1, BL + 1)
        # packed smalls: rcoln | rcolp | maskc | b1t | bc1t | cvec(row 0)
        pack = np.zeros((P, BL * IC * 3 * 2 + BL * IC + 2 * HC + BL + 1),
                        np.float32)
        nc3 = BL * IC * 3
        pack[:, 0:nc3] = -rcolp
        pack[:, nc3:2 * nc3] = rcolp
        pack[:, 2 * nc3:2 * nc3 + BL * IC] = maskc
        o = 2 * nc3 + BL * IC
        pack[:, o:o + HC] = b1t
        pack[:, o + HC:o + 2 * HC] = bc1t
        pack[0, o + 2 * HC:o + 2 * HC + BL + 1] = cvec[0]
        packr = np.concatenate([w2t, wc2t], axis=1)
        in_maps.append({
            "rept": rept,
            "w1h": w1h, "wc1h": wc1h,
            "pack": pack, "packr": np.ascontiguousarray(packr),
        })
    return in_maps


def kernel(**inputs) -> np.ndarray:
    nc = _get_program()
    in_maps = _host_prep(inputs)
    res = None
    last_err = None
    for attempt in range(3):
        try:
            res = bass_utils.run_bass_kernel_spmd(
                nc, in_maps, core_ids=list(range(NCORES)))
            break
        except Exception as e:  # transient NRT_EXEC_UNIT faults have been seen
            last_err = e
            import time
            time.sleep(2.0)
            try:
                import jax
                jax.clear_backends()
            except Exception:
                pass
    if res is None:
        raise last_err
    out = np.concatenate([res.results[c]["out"][0] for c in range(NCORES)])
    return out.reshape(B, 1).astype(np.float32)


# revision 79
# speedup vs baseline: 1.0477x; 1.0061x over previous
"""Trainium2 Bass kernel for the EnergyCoulomb problem.

Reference computation (per molecule, B=32, N=512, D=1024, H=512):
  y  = sum_atoms(mask * (ssp(rep @ W1 + b1) @ W2 + b2))           atomwise MLP + pool
  q  = ssp(rep @ Wc1 + bc1) @ Wc2 + bc2                           charge net
  e  = sum_{i!=j} q_i q_j (1e-5 + |R_i - R_j|)^-2 * mask_i mask_j coulomb term
  out = y + e

Sharding: data-parallel over molecules, 4 molecules per core on 8 cores,
weights replicated.

Key design points (vs the 131.7us first-generation kernel):
  * rep is pre-transposed on the host into [128, KD*N] per molecule: the PE
    never transposes it and nothing copies transposes out of PSUM.
  * The DMA pool is effectively one serial ~330GB/s resource and every
    trigger costs ~630ns of its host sequencer's time: ALL transfers ride
    the SP ring (never the ACT ring, which would clog ACT's instruction
    queue), in consumption order, in 4KB-per-partition chunks, with all
    small inputs packed into two tensors. The z matmuls run k-major in
    half-sets so each arriving chunk immediately feeds matmuls and each
    half-set's Exps fire early, halving the PSUM-rotation stall at set
    boundaries.
  * (1e-5 + dist)^-2 is approximated by 1/d2 (max rel err ~8e-4 on the
    closest pairs, far under the 2e-2 gate): the entire sqrt chain
    (Ln, Exp, +1e-5, square) disappears. The diagonal d2 is exactly zero
    by construction; reciprocal gives inf there and affine_select
    replaces it with 0 before any consumer.
  * R rows and mask rows are built ON DEVICE (PE column->row transposes,
    DVE copy out of PSUM — GPSIMD cannot read PSUM — then Pool
    partition_broadcast) from column-spread inputs, keeping row broadcasts
    out of the serial DMA stream entirely.
  * d2 squares: x,y coords on ACT (Square with bias=-coord), z coord on
    Pool(sub)+DVE(mul) to balance engines; softplus is split so the Exps
    (which free PSUM banks the next set waits on) always sit at the head
    of the ACT queue, while the fused [128, HC*N] Ln runs one set later.
  * Pairwise chains and row builds are staggered through the schedule so
    their ACT/DVE/Pool stages fill idle windows without head-of-line
    blocking the Exps; the last chain gets dedicated d2/tmp buffers so its
    reciprocal lands before the final epilogue needs it.

ssp(x) = softplus(x) - ln2 is folded as softplus on device plus a host-side
constant shift c = b - ln2 * sum(W_layer2) applied at the pooled level.
"""

import numpy as np

import concourse.bass as bass
import concourse.bacc as bacc
import concourse.mybir as mybir
import concourse.tile as tile
from concourse import bass_utils
from concourse.masks import make_identity

LOG2 = float(np.log(2.0))

B, N, D, H = 32, 512, 1024, 512
NCORES = 8
BL = B // NCORES          # molecules per core
P = 128                   # partitions
KD = D // P               # 8 K-chunks over D
HC = H // P               # 4 h-chunks over H
IC = N // P               # 4 i-chunks over atoms

f32 = mybir.dt.float32
f32r = mybir.dt.float32r
AF = mybir.ActivationFunctionType
ALU = mybir.AluOpType
AX = mybir.AxisListType

_CACHE = {}

# Every ACT function this kernel uses (Exp, Ln, Square, Copy) lives in the
# "natural_log_exp_and_others" table set. Bacc's table chooser is
# greedy-first-match; emptying every other set (order preserved, so
# act_func_set_id indices stay valid) pins the chooser to the combined set:
# one table load for the whole kernel.
_ONE_TABLE = "natural_log_exp_and_others"


def _gat_one_table(arch):
    from concourse.hw_specs import get_activation_tables
    tabs = get_activation_tables(arch)
    assert _ONE_TABLE in tabs
    return {n: (fns if n == _ONE_TABLE else set()) for n, fns in tabs.items()}


def _build_program():
    bacc.get_activation_tables = _gat_one_table
    nc = bacc.Bacc("TRN2", target_bir_lowering=False, debug=False,
                   enable_asserts=False)

    # host-pretransposed rep: rt[b][p, k*N+n] = rep[b, n, k*128+p]
    rept_d = nc.dram_tensor("rept", [BL, P, KD * N], f32r, kind="ExternalInput").ap()
    # host-chunked weights: w1h[p, k*H+h] = W1[k*128+p, h]
    w1h_d = nc.dram_tensor("w1h", [P, KD * H], f32r, kind="ExternalInput").ap()
    wc1h_d = nc.dram_tensor("wc1h", [P, KD * H], f32r, kind="ExternalInput").ap()
    # all small inputs packed into two tensors (each DMA trigger costs
    # ~630ns of ring-sequencer time, so 8 separate smalls would delay the
    # weight stream by ~4us): packed f32 columns are
    #   rcoln[48] | rcolp[48] | maskc[16] | b1t[4] | bc1t[4] | cvec row[5]
    # where rcoln[p, (b*IC+ic)*3+c] = -R[b, ic*128+p, c] (bias/scalar
    # operands), rcolp the same un-negated (row-transpose source), and
    # maskc[p, b*IC+ic] = mask[b, ic*128+p].
    NPK = BL * IC * 3 * 2 + BL * IC + 2 * HC + (BL + 1)
    pack_d = nc.dram_tensor("pack", [P, NPK], f32, kind="ExternalInput").ap()
    packr_d = nc.dram_tensor("packr", [P, 2 * HC], f32r, kind="ExternalInput").ap()
    out_d = nc.dram_tensor("out", [1, BL], f32, kind="ExternalOutput").ap()

    with tile.TileContext(nc) as tc:
        with tc.tile_pool(name="singles", bufs=1) as singles, \
             tc.tile_pool(name="work", bufs=1) as work, \
             tc.tile_pool(name="ps", bufs=1, space="PSUM") as ps:

            ident32 = singles.tile([1, 1], f32, tag="ident32")
            nc.vector.memset(ident32, 1.0)
            ident = singles.tile([P, P], f32, tag="ident")
            make_identity(nc, ident)
            ones_col = singles.tile([P, 1], f32, tag="ones_col")
            nc.vector.memset(ones_col, 1.0)

            # ---- input streaming ----
            # Everything rides the SP-sequencer HWDGE ring in consumption
            # order: each trigger costs ~630ns of its host sequencer, so
            # putting any of these on the ACT ring would clog the ACT
            # engine's instruction queue. 4KB chunks keep the trigger rate
            # comfortably ahead of the ~330GB/s serial transfer rate while
            # still drip-feeding the k-major matmuls.
            def ring_dma(dst, src):
                nc.sync.dma_start(dst, src)

            rt_sb = []
            for b in range(BL):
                rt_sb.append(work.tile([P, KD * N], f32r, tag="rt", bufs=BL - 1,
                                       name=f"rt{b}"))
            wc1_sb = singles.tile([P, KD * H], f32r, tag="wc1h")
            w1_sb = singles.tile([P, KD * H], f32r, tag="w1h")

            pack = singles.tile([P, NPK], f32, tag="pack")
            nc.sync.dma_start(pack, pack_d)
            packr = singles.tile([P, 2 * HC], f32r, tag="packr")
            nc.sync.dma_start(packr, packr_d)
            NC3 = BL * IC * 3
            rcoln = pack[:, 0:NC3]
            rcolp = pack[:, NC3:2 * NC3]
            maskc = pack[:, 2 * NC3:2 * NC3 + BL * IC]
            _o = 2 * NC3 + BL * IC
            b1t = pack[:, _o:_o + HC]
            bc1t = pack[:, _o + HC:_o + 2 * HC]
            cvec = pack[0:1, _o + 2 * HC:_o + 2 * HC + BL + 1]
            w2t = packr[:, 0:HC]
            wc2t = packr[:, HC:2 * HC]

            # per-molecule consumption order: wc1 (mol0 charge), rt0, w1
            # (mol0 y), rt1, rt2, rt3 — in 2-k (4KB/partition) chunks
            for k in range(0, KD, 2):
                ring_dma(wc1_sb[:, k * H:(k + 2) * H], wc1h_d[:, k * H:(k + 2) * H])
                ring_dma(rt_sb[0][:, k * N:(k + 2) * N], rept_d[0][:, k * N:(k + 2) * N])
            for k in range(0, KD, 2):
                ring_dma(w1_sb[:, k * H:(k + 2) * H], w1h_d[:, k * H:(k + 2) * H])
            for b in range(1, BL):
                for k in range(0, KD, 2):
                    ring_dma(rt_sb[b][:, k * N:(k + 2) * N],
                             rept_d[b][:, k * N:(k + 2) * N])

            res = singles.tile([1, BL], f32, tag="res")

            # ---- device-side row builds (replaces row-broadcast DMAs) ----
            xjb_tiles = {}
            mrows = {}

            def build_xjb(b):
                # GPSIMD cannot read PSUM: transpose -> DVE copy to an SBUF
                # row -> Pool broadcast
                xjb = work.tile([P, 3, N], f32, tag="xjb", bufs=2)
                xrow = work.tile([1, 3, N], f32, tag="xrow", bufs=1)
                for c in range(3):
                    xp = ps.tile([1, N], f32, tag="xrow_ps", bufs=1)
                    for ic in range(IC):
                        nc.tensor.transpose(
                            xp[0:1, ic * P:(ic + 1) * P],
                            rcolp[:, (b * IC + ic) * 3 + c:(b * IC + ic) * 3 + c + 1],
                            ident)
                    nc.vector.tensor_copy(xrow[0:1, c, :], xp)
                nc.gpsimd.partition_broadcast(xjb, xrow)
                xjb_tiles[b] = xjb

            def build_mrow(b):
                mp = ps.tile([1, N], f32, tag="xrow_ps", bufs=1)
                for ic in range(IC):
                    nc.tensor.transpose(
                        mp[0:1, ic * P:(ic + 1) * P],
                        maskc[:, b * IC + ic:b * IC + ic + 1],
                        ident)
                m = singles.tile([1, N], f32, tag=f"mrow_{b}")
                nc.vector.tensor_copy(m, mp)
                mrows[b] = m

            # ---- pairwise chain: rb[p, ic, j] = 1/d2_(128ic+p),j  (0 diag) --
            # Split into an ACT-square part and a DVE/Pool part so each can
            # be placed independently in its engine's in-order queue.
            def chain_sq(b):
                # two buffer pairs alternating by parity: chain k only
                # serializes against chain k-2, so chains 2/3 can run a full
                # stage earlier and their reciprocals clear the tail
                xjb = xjb_tiles[b]
                tag_sfx = "AB"[b % 2]
                d2b = work.tile([P, IC, N], f32, tag="d2b" + tag_sfx, bufs=1)
                tmpb = work.tile([P, IC, N], f32, tag="tmpb" + tag_sfx, bufs=1)
                for ic in range(IC):
                    col = (b * IC + ic) * 3
                    nc.scalar.activation(d2b[:, ic, :], xjb[:, 0, :], AF.Square,
                                         bias=rcoln[:, col + 0:col + 1])
                    nc.scalar.activation(tmpb[:, ic, :], xjb[:, 1, :], AF.Square,
                                         bias=rcoln[:, col + 1:col + 2])
                return d2b, tmpb

            def chain_rest(b, d2b, tmpb, dve_subs=False):
                xjb = xjb_tiles[b]
                nc.vector.tensor_tensor(d2b, d2b, tmpb, op=ALU.add)
                # z coord: subtract on Pool (DVE for the last chain, whose
                # latency is on the final epilogue's critical path), square +
                # accumulate on DVE
                sub_eng = nc.vector if dve_subs else nc.gpsimd
                for ic in range(IC):
                    col = (b * IC + ic) * 3
                    sub_eng.tensor_scalar(tmpb[:, ic, :], xjb[:, 2, :],
                                          rcoln[:, col + 2:col + 3], None,
                                          op0=ALU.add)
                nc.vector.tensor_mul(tmpb, tmpb, tmpb)
                nc.vector.tensor_tensor(d2b, d2b, tmpb, op=ALU.add)
                nc.vector.reciprocal(d2b, d2b)
                rb = work.tile([P, IC, N], f32r, tag="rb", bufs=2)
                nc.gpsimd.affine_select(
                    out=rb, in_=d2b, compare_op=ALU.not_equal, fill=0.0,
                    base=0, pattern=[[P, IC], [-1, N]], channel_multiplier=1)
                return rb

            rb_tiles = {}
            h_tiles = {}
            chain_mid = {}

            # ---- per-molecule MLP set, two phases ----
            # z-phase: k-major z matmuls, then the Exps IMMEDIATELY (they
            # free the PSUM slots the next set's matmuls are waiting on).
            # ln-phase: the (fused, bias-free) Ln runs later, in the next
            # set's z-window, so it never blocks exps in the in-order ACT
            # queue.
            ez_tiles = {}

            def emit_mlp_z(b, wset):
                # half-sets (hc 0,1 then hc 2,3): the first half's exps fire
                # ~3.4us before the set's end, so the next set's matmuls wait
                # only on the second half's exps (halved boundary stall).
                w_sb, bias = (wc1_sb, bc1t) if wset == "q" else (w1_sb, b1t)
                zts = [ps.tile([P, N], f32, tag="z", bufs=5,
                               name=f"z_{b}_{wset}_{hc}") for hc in range(HC)]
                ez = work.tile([P, HC, N], f32, tag="ez", bufs=2)
                for half in range(2):
                    hcs = (0, 1) if half == 0 else (2, 3)
                    for k in range(KD):
                        for hc in hcs:
                            nc.tensor.matmul(
                                zts[hc],
                                lhsT=w_sb[:, k * H + hc * P:k * H + (hc + 1) * P],
                                rhs=rt_sb[b][:, k * N:(k + 1) * N],
                                start=(k == 0), stop=(k == KD - 1))
                    for hc in hcs:
                        nc.scalar.activation(ez[:, hc, :], zts[hc], AF.Exp,
                                             bias=bias[:, hc:hc + 1])
                ez_tiles[(b, wset)] = ez

            def emit_mlp_ln(b, wset, halves=False):
                ez = ez_tiles.pop((b, wset))
                h = work.tile([P, HC, N], f32r, tag=f"h_{wset}", bufs=2)
                if halves:
                    nc.scalar.activation(h[:, 0:2, :], ez[:, 0:2, :], AF.Ln,
                                         bias=ones_col[:, 0:1])
                    nc.scalar.activation(h[:, 2:4, :], ez[:, 2:4, :], AF.Ln,
                                         bias=ones_col[:, 0:1])
                else:
                    nc.scalar.activation(h, ez, AF.Ln, bias=ones_col[:, 0:1])
                h_tiles[(b, wset)] = h

            # ---- per-molecule epilogue (charge front / finish split) ----
            epi_front = {}

            def emit_epi_front(b):
                """q row + q columns — needs only ln(b, q) and the mask."""
                hq = h_tiles.pop((b, "q"))
                q_ps = ps.tile([1, N], f32, tag="row_ps", bufs=2)
                for hc in range(HC):
                    nc.tensor.matmul(q_ps,
                                     lhsT=wc2t[:, hc:hc + 1],
                                     rhs=hq[:, hc, :],
                                     start=(hc == 0), stop=(hc == HC - 1))
                qrow = work.tile([1, N], f32, tag="qrow", bufs=1)
                nc.vector.tensor_scalar(qrow, q_ps, cvec[0:1, BL:BL + 1], None,
                                        op0=ALU.add)
                nc.vector.tensor_mul(qrow, qrow, mrows[b])

                qc_ps = ps.tile([P, IC], f32, tag="row_ps", bufs=2)
                for ic in range(IC):
                    nc.tensor.transpose(qc_ps[:, ic:ic + 1],
                                        qrow[:, ic * P:(ic + 1) * P],
                                        ident32[0:1, 0:1])
                qc = work.tile([P, IC], f32r, tag="qc", bufs=2)
                nc.vector.tensor_copy(qc, qc_ps)
                epi_front[b] = (qrow, qc)

            epi_mid = {}

            def emit_epi_mid(b):
                """coulomb t-matvec + e reduction — needs rb and the front."""
                qrow, qc = epi_front.pop(b)
                rb = rb_tiles.pop(b)
                t_ps = ps.tile([1, N], f32, tag="row_ps", bufs=2)
                for ic in range(IC):
                    nc.tensor.matmul(t_ps,
                                     lhsT=qc[:, ic:ic + 1],
                                     rhs=rb[:, ic, :],
                                     start=(ic == 0), stop=(ic == IC - 1))
                scr_e = work.tile([1, N], f32, tag="scr", bufs=1)
                nc.vector.tensor_mul(scr_e, t_ps, qrow)
                e_sb = work.tile([1, 1], f32, tag="e_sb", bufs=2)
                nc.vector.reduce_sum(e_sb, scr_e, axis=AX.X)
                epi_mid[b] = e_sb

            def emit_epilogue(b):
                if b not in epi_front and b not in epi_mid:
                    emit_epi_front(b)
                if b not in epi_mid:
                    emit_epi_mid(b)
                e_sb = epi_mid.pop(b)
                h1 = h_tiles.pop((b, "y"))
                yi_ps = ps.tile([1, N], f32, tag="row_ps", bufs=2)
                for hc in range(HC):
                    nc.tensor.matmul(yi_ps,
                                     lhsT=w2t[:, hc:hc + 1],
                                     rhs=h1[:, hc, :],
                                     start=(hc == 0), stop=(hc == HC - 1))
                scr_y = work.tile([1, N], f32, tag="scr", bufs=1)
                nc.vector.tensor_mul(scr_y, yi_ps, mrows[b])
                ysum = work.tile([1, 1], f32, tag="ysum", bufs=2)
                nc.vector.reduce_sum(ysum, scr_y, axis=AX.X)
                # res[b] = (ysum + cvec_b) + e_sb in a single two-op pass
                nc.vector.tensor_scalar(res[:, b:b + 1], ysum,
                                        cvec[0:1, b:b + 1], e_sb,
                                        op0=ALU.add, op1=ALU.add)

            # ---- schedule ----
            # Row builds + all four pairwise chains go first: their ACT
            # squares fill the DMA-drip phase where ACT is otherwise idle,
            # and every rb is ready long before its epilogue.
            build_xjb(0)
            chain_mid[0] = chain_sq(0)
            emit_mlp_z(0, "q")
            build_xjb(1)                      # PE transposes fill the boundary
            rb_tiles[0] = chain_rest(0, *chain_mid.pop(0))
            emit_mlp_z(0, "y")
            chain_mid[1] = chain_sq(1)
            emit_mlp_ln(0, "q")
            build_mrow(0)
            emit_epi_front(0)
            emit_mlp_z(1, "q")
            build_xjb(2)
            rb_tiles[1] = chain_rest(1, *chain_mid.pop(1))
            chain_mid[2] = chain_sq(2)
            emit_epi_mid(0)
            rb_tiles[2] = chain_rest(2, *chain_mid.pop(2))
            emit_mlp_ln(0, "y")
            emit_epilogue(0)
            emit_mlp_z(1, "y")
            build_xjb(3)
            chain_mid[3] = chain_sq(3)
            emit_mlp_ln(1, "q")
            build_mrow(1)
            emit_epi_front(1)
            emit_epi_mid(1)
            rb_tiles[3] = chain_rest(3, *chain_mid.pop(3))
            emit_mlp_ln(1, "y")
            emit_mlp_z(2, "q")
            emit_epilogue(1)
            emit_mlp_z(2, "y")
            emit_mlp_ln(2, "q")
            build_mrow(2)
            emit_epi_front(2)
            emit_epi_mid(2)
            emit_mlp_ln(2, "y")
            emit_mlp_z(3, "q")
            emit_epilogue(2)
            emit_mlp_ln(3, "q")
            build_mrow(3)
            emit_epi_front(3)
            emit_mlp_z(3, "y")
            emit_mlp_ln(3, "y", halves=True)
            emit_epilogue(3)

            nc.sync.dma_start(out_d, res)

    nc.compile()
    return nc


def _get_program():
    if "nc" not in _CACHE:
        _CACHE["nc"] = _build_program()
    return _CACHE["nc"]


def _host_prep(inputs):
    """Build per-core in_maps from full inputs."""
    rep = np.asarray(inputs["representation"], np.float32)
    R = np.asarray(inputs["R"], np.float32)
    mask = np.asarray(inputs["atom_mask"], np.float32)
    W1 = np.asarray(inputs["W1"], np.float32)
    b1 = np.asarray(inputs["b1"], np.float32)
    W2 = np.asarray(inputs["W2"], np.float32)
    b2 = np.asarray(inputs["b2"], np.float32)
    Wc1 = np.asarray(inputs["Wc1"], np.float32)
    bc1 = np.asarray(inputs["bc1"], np.float32)
    Wc2 = np.asarray(inputs["Wc2"], np.float32)
    bc2 = np.asarray(inputs["bc2"], np.float32)

    # w1h[p, k*H + h] = W1[k*128+p, h]
    w1h = np.ascontiguousarray(
        W1.reshape(KD, P, H).transpose(1, 0, 2).reshape(P, KD * H))
    wc1h = np.ascontiguousarray(
        Wc1.reshape(KD, P, H).transpose(1, 0, 2).reshape(P, KD * H))
    b1t = np.ascontiguousarray(b1.reshape(HC, P).T)
    bc1t = np.ascontiguousarray(bc1.reshape(HC, P).T)
    w2t = np.ascontiguousarray(W2[:, 0].reshape(HC, P).T)
    wc2t = np.ascontiguousarray(Wc2[:, 0].reshape(HC, P).T)
    c2 = np.float32(b2[0] - LOG2 * W2.sum(dtype=np.float64))
    cq = np.float32(bc2[0] - LOG2 * Wc2.sum(dtype=np.float64))

    in_maps = []
    for c in range(NCORES):
        sl = slice(c * BL, (c + 1) * BL)
        Rb = R[sl]                                   # [BL, N, 3]
        # rcolp[p, (b*IC+ic)*3 + c] = R[b, ic*128+p, c]; rcoln negated
        rcolp = np.ascontiguousarray(
            Rb.reshape(BL, IC, P, 3).transpose(2, 0, 1, 3).reshape(P, BL * IC * 3))
        # maskc[p, b*IC+ic] = mask[b, ic*128+p]
        maskc = np.ascontiguousarray(
            mask[sl].reshape(BL, IC, P).transpose(2, 0, 1).reshape(P, BL * IC))
        # rept[b][p, k*N + n] = rep[b, n, k*128+p]
        rept = np.ascontiguousarray(
            rep[sl].reshape(BL, N, KD, P).transpose(0, 3, 2, 1).reshape(BL, P, KD * N))
        cvec = np.concatenate(
            [c2 * mask[sl].sum(axis=1, dtype=np.float32), [cq]]
        ).astype(np.float32).reshape(1, BL + 1)
        # packed smalls: rcoln | rcolp | maskc | b1t | bc1t | cvec(row 0)
        pack = np.zeros((P, BL * IC * 3 * 2 + BL * IC + 2 * HC + BL + 1),
                        np.float32)
        nc3 = BL * IC * 3
        pack[:, 0:nc3] = -rcolp
        pack[:, nc3:2 * nc3] = rcolp
        pack[:, 2 * nc3:2 * nc3 + BL * IC] = maskc
        o = 2 * nc3 + BL * IC
        pack[:, o:o + HC] = b1t
        pack[:, o + HC:o + 2 * HC] = bc1t
        pack[0, o + 2 * HC:o + 2 * HC + BL + 1] = cvec[0]
        packr = np.concatenate([w2t, wc2t], axis=1)
        in_maps.append({
            "rept": rept,
            "w1h": w1h, "wc1h": wc1h,
            "pack": pack, "packr": np.ascontiguousarray(packr),
        })
    return in_maps


def kernel(**inputs) -> np.ndarray:
    nc = _get_program()
    in_maps = _host_prep(inputs)
    res = None
    last_err = None
    for attempt in range(3):
        try:
            res = bass_utils.run_bass_kernel_spmd(
                nc, in_maps, core_ids=list(range(NCORES)))
            break
        except Exception as e:  # transient NRT_EXEC_UNIT faults have been seen
            last_err = e
            import time
            time.sleep(2.0)
            try:
                import jax
                jax.clear_backends()
            except Exception:
                pass
    if res is None:
        raise last_err
    out = np.concatenate([res.results[c]["out"][0] for c in range(NCORES)])
    return out.reshape(B, 1).astype(np.float32)
